# revision 32
# baseline (speedup 1.0000x reference)
"""Self-contained Trainium2 Bass kernel for nn_MixedNet_61753039781957.

MixedNet: 4-layer MLP, B=4096, D_in=1024, H=4096, D_out=1024.
  h = x
  for (W, a) in ((W0,a0),(W1,a1),(W2,a2)):
      z = h @ W
      h = a * concat([sin(z[:, :2048]), tanh(z[:, 2048:3072]), log(z[:, 3072:]**2)])
  y = h @ W3

Strategy (data-parallel, no collectives):
  - Shard batch across 8 NeuronCores (512 rows each), replicate weights.
  - Keep activations TRANSPOSED on-chip: hT[hidden, batch] so each matmul is
    psum[128(nblk), 512(batch)] += Wblk[128k, 128n].T @ hT[128k, 512] with the
    weight block as the stationary operand (no on-chip transposes anywhere).
  - Matmuls run in float32r (fp32 storage, FP22 multiply) -> full PE rate for
    moving dim >= 256.
  - alpha scaling is folded into the following layer's weight rows on the
    host; 1/(2pi) is folded into sin-segment weight columns so the on-chip sin
    path is: k = round(u) (DVE magic-number trick), f = u - k, ACT
    Sin(scale=2pi) -- the raw ACT Sin LUT is only accurate for |x| < ~3.9.
    tanh: single ACT op. log(z^2): ACT Square then ACT Ln (Square is in every
    ACT table set, so PSUM banks release before the Ln table switch).
  - Weights are pre-tiled on the host into exact DMA consumption order
    (contiguous [128, 512] blocks = 4 PSUM banks per n-group, so two groups
    double-buffer across the 8 banks); all weight DMAs issue from the SP HWDGE
    queue (the ACT queue stalls behind activation bursts), x/y use the ACT
    queue. ~10 dummy matmuls at the start keep the PE HAM clock gate warm
    through the initial DMA ramp.
    Measured: ~604us HW exec per core (2560 f32r matmuls at ~227ns issue
    spacing, ~91% of the 78.6 TF/s PE peak), rel l2 error ~2.5e-3 vs the f32
    reference (device-vs-cpu reference noise floor is ~1.9e-4).
"""

import sys
import types

sys.path.insert(0, "/opt/trn_rl_repo")

import numpy as np

NCORES = 8
B, D_IN, H, D_OUT = 4096, 1024, 4096, 1024
BS = B // NCORES  # batch shard per core
GW = 512          # n-group width (4 blocks of 128 hidden units -> 4 PSUM banks,
                  # so two groups double-buffer across the 8 PSUM banks)
SEG_SIN_END = 2048 // 128   # block idx < 16 -> sin
SEG_TANH_END = 3072 // 128  # block idx < 24 -> tanh, else ln


def _install_axon_hooks():
    """Provide antenv.axon_hooks (missing in this image) so that
    run_bass_kernel_spmd(trace=True) can capture NTFF profiles."""
    try:
        import antenv
    except ImportError:
        return
    if "antenv.axon_hooks" in sys.modules:
        return
    mod = types.ModuleType("antenv.axon_hooks")
    hook = [None]
    mod.set_axon_ntff_profile_hook = lambda h: hook.__setitem__(0, h)
    mod.get_axon_ntff_profile_hook = lambda: hook[0]
    sys.modules["antenv.axon_hooks"] = mod
    antenv.axon_hooks = mod
    try:
        from trn_agent_boot.trn_boot import _ntff_profile_via_ctypes

        h = _ntff_profile_via_ctypes("/opt/axon/libaxon_pjrt.so")
        if h is not None:
            mod.set_axon_ntff_profile_hook(h)
    except Exception:
        pass


def _patch_tile_drain():
    """walrus CoreV3 codegen rejects instructions with >4 semaphore waits; the
    TileContext tail drain collects one wait per live semaphore. Spread the
    waits over several consecutive drain instructions."""
    import concourse.tile as tile_mod
    from concourse import mybir
    from concourse.vector_clock import ScopedClock

    if getattr(tile_mod.TileContext, "_ant_drain_split", False):
        return

    MAXW = 4

    def _drain_and_barrier(self, tick_clock, wait_clock):
        nc = self.nc
        drain_inst = nc.sync.drain()
        wait_clock.add_sem_waits(
            drain_inst.ins, ScopedClock({None: tick_clock.global_clock})
        )
        si = drain_inst.ins.sync_info
        if si is not None and si.on_wait and len(si.on_wait) > MAXW:
            waits = list(si.on_wait)
            updates = list(si.on_update or [])
            drain_inst.ins.sync_info = mybir.SyncInfo(
                on_wait=waits[:MAXW], on_update=[]
            )
            rest = waits[MAXW:]
            while rest:
                chunk, rest = rest[:MAXW], rest[MAXW:]
                d = mybir.InstDrain(
                    name=nc.get_next_instruction_name(),
                    ins=[],
                    outs=[],
                    bass_is_fusable=False,
                )
                d.engine = nc.sync.engine
                d.sync_info = mybir.SyncInfo(
                    on_wait=chunk, on_update=updates if not rest else []
                )
                nc.sync.add_instruction(d)
        nc.all_engine_barrier()
        assert self.sems is not None
        popped = nc._tile_sem_poison_stack.pop()
        assert popped is self._sem_poison
        # NOTE: the stock tile epilogue emits a second all-engine barrier
        # after the semaphore clear; nothing executes after it and NRT waits
        # for every engine stream to end anyway, so skip it (~3-5us tail).
        nc.clear_and_free_semaphores(list(self.sems.allocated().values()))

    tile_mod.TileContext._drain_and_barrier = _drain_and_barrier
    tile_mod.TileContext._ant_drain_split = True


def _split_excess_waits(nc, maxw=1, maxw_mm=1):
    """walrus CoreV3 setupSyncWait rejects instructions with too many sem
    waits (4 generally; fewer for self-loading-weights Matmult). Spill excess
    waits onto NoOps inserted just before the instruction on the same engine
    (same semantics: the engine stream is serial)."""
    from concourse import mybir

    def limit_of(inst):
        return maxw_mm if isinstance(inst, mybir.InstMatmult) else maxw

    for fn in nc.m.functions:
        for bb in fn.blocks:
            need = any(
                getattr(i, "sync_info", None)
                and i.sync_info.on_wait
                and len(i.sync_info.on_wait) > limit_of(i)
                for i in bb.instructions
            )
            if not need:
                continue
            new = []
            for inst in bb.instructions:
                lim = limit_of(inst)
                si = getattr(inst, "sync_info", None)
                if si is not None and si.on_wait and len(si.on_wait) > lim:
                    waits = list(si.on_wait)
                    head, tail = waits[:-lim] if lim else waits, waits[-lim:] if lim else []
                    while head:
                        chunk, head = head[:maxw], head[maxw:]
                        nop = mybir.InstNoOp(
                            name=nc.get_next_instruction_name(),
                            ins=[],
                            outs=[],
                            sync_info=mybir.SyncInfo(on_wait=chunk, on_update=[]),
                        )
                        nop.engine = inst.engine
                        new.append(nop)
                    inst.sync_info = mybir.SyncInfo(
                        on_wait=tail, on_update=si.on_update
                    )
                new.append(inst)
            bb.instructions = new


def build_bass(bs=BS, d_in=D_IN, h=H, d_out=D_OUT, w_bufs=16, debug=False):
    """Build the per-core Bass program (same NEFF on all cores, SPMD)."""
    _install_axon_hooks()
    _patch_tile_drain()

    import concourse.bass as bass
    import concourse.tile as tile
    from concourse import mybir

    f32 = mybir.dt.float32
    f32r = mybir.dt.float32r
    AF = mybir.ActivationFunctionType

    kt = [d_in // 128, h // 128, h // 128, h // 128]      # k tiles per layer
    ng = [h // GW, h // GW, h // GW, d_out // GW]          # n groups per layer

    nc = bass.Bass()
    xT = nc.declare_dram_parameter("xT", [d_in, bs], f32, isOutput=False)
    w_d = [
        nc.declare_dram_parameter(
            f"w{i}", [ng[i] * kt[i], 128, GW], f32, isOutput=False
        )
        for i in range(4)
    ]
    yT = nc.declare_dram_parameter("yT", [d_out, bs], f32, isOutput=True)
    dbg_d = None
    if debug:
        dbg_d = [
            nc.declare_dram_parameter(f"h{i}T", [h, bs], f32, isOutput=True)
            for i in (1, 2, 3)
        ]

    with tile.TileContext(nc) as tc:
        with (
            tc.tile_pool(name="xp", bufs=d_in // 128) as xp,
            tc.tile_pool(name="ha", bufs=h // 128) as ha,
            tc.tile_pool(name="hb", bufs=h // 128) as hb,
            tc.tile_pool(name="wp", bufs=w_bufs) as wp,
            tc.tile_pool(name="tp", bufs=4) as tp,
            tc.tile_pool(name="yp", bufs=4) as yp,
            tc.tile_pool(name="ps", bufs=8, space="PSUM") as ps,
        ):
            # Warm the PE HAM clock gate during the initial DMA ramp: the gate
            # only opens (1.2 -> 2.4 GHz) after ~3.4us of sustained PE
            # activity, so burn that time on dummy matmuls with no DMA deps.
            warm = wp.tile([128, bs], f32r, tag="warm", bufs=1)
            nc.vector.memset(warm.bitcast(f32), 1.0)
            wps = ps.tile([128, bs], f32, tag="ps")
            for i in range(8):
                nc.tensor.matmul(
                    wps, lhsT=warm[:, :128], rhs=warm,
                    start=(i == 0), stop=(i == 7),
                )

            # load x shard (transposed) into SBUF, alternating the two HWDGE
            # queues (SP + ACT) so x and the weight stream run in parallel
            h_in = []
            for k in range(d_in // 128):
                xt = xp.tile([128, bs], f32r, tag="x")
                nc.scalar.dma_start(out=xt, in_=xT[k * 128:(k + 1) * 128, :].bitcast(f32r))
                h_in.append(xt)

            def act_fun_for_block(blk, nblocks, final):
                if final:
                    return "copy"
                # segment layout scales with h for small test configs
                sin_end = (nblocks * 2048) // 4096
                tanh_end = (nblocks * 3072) // 4096
                if blk < sin_end:
                    return "sin"
                if blk < tanh_end:
                    return "tanh"
                return "ln"

            for layer in range(4):
                final = layer == 3
                out_pool = yp if final else (ha, hb, ha)[layer]
                out_tag = "y" if final else f"h{(ha, hb, ha)[layer].name}"
                h_out = []
                jn = GW // 128
                for g in range(ng[layer]):
                    psums = []
                    for j in range(jn):
                        pt = ps.tile([128, bs], f32, tag="ps", name=f"ps_l{layer}_g{g}_{j}")
                        psums.append(pt)
                    for k in range(kt[layer]):
                        wt = wp.tile([128, GW], f32r, tag="w", name=f"w_l{layer}_g{g}_k{k}")
                        # weights always via SP: the ACT engine's instruction
                        # stream stalls on activation bursts + table loads,
                        # which would delay DMA issue and starve the PE
                        nc.sync.dma_start(out=wt, in_=w_d[layer][g * kt[layer] + k, :, :].bitcast(f32r))
                        for j in range(jn):
                            nc.tensor.matmul(
                                psums[j],
                                lhsT=wt[:, j * 128:(j + 1) * 128],
                                rhs=h_in[k],
                                start=(k == 0),
                                stop=(k == kt[layer] - 1),
                            )
                    # pass 1: drain each PSUM bank ASAP with an op that is
                    # valid in ANY act table set (Square) or on DVE, so the
                    # next group's matmuls are never gated on the Ln
                    # table-load; pass 2 runs the table-set-sensitive ops.
                    pre = {}
                    for j in range(jn):
                        blk = g * jn + j
                        fun = act_fun_for_block(blk, ng[layer] * jn, final)
                        if fun == "sin":
                            MAGIC = float(np.float32(1.5 * 2 ** 23))
                            ktile = tp.tile([128, bs], f32, tag="t", name=f"k_l{layer}_b{blk}")
                            nc.vector.tensor_scalar(
                                out=ktile, in0=psums[j],
                                scalar1=MAGIC, scalar2=MAGIC,
                                op0=mybir.AluOpType.add,
                                op1=mybir.AluOpType.subtract,
                            )
                            ftile = tp.tile([128, bs], f32, tag="t2", name=f"f_l{layer}_b{blk}")
                            nc.vector.tensor_tensor(
                                out=ftile, in0=psums[j], in1=ktile,
                                op=mybir.AluOpType.subtract,
                            )
                            pre[j] = ftile
                        elif fun == "ln":
                            tt = tp.tile([128, bs], f32, tag="t", name=f"t_l{layer}_b{blk}")
                            nc.scalar.activation(tt, psums[j], AF.Square)
                            pre[j] = tt
                    for j in range(jn):
                        blk = g * jn + j
                        fun = act_fun_for_block(blk, ng[layer] * jn, final)
                        ot = out_pool.tile(
                            [128, bs], f32 if final else f32r, tag=out_tag,
                            name=f"o_l{layer}_b{blk}"
                        )
                        if fun == "sin":
                            # psum held u = z/(2pi) (folded into the weight
                            # columns on the host); pre[j] = u - round(u),
                            # so sin(2pi*pre[j]) = sin(z).
                            nc.scalar.activation(
                                ot, pre[j], AF.Sin, scale=float(2 * np.pi)
                            )
                        elif fun == "tanh":
                            nc.scalar.activation(ot, psums[j], AF.Tanh)
                        elif fun == "ln":
                            nc.scalar.activation(ot, pre[j], AF.Ln)
                        else:  # final layer: drain PSUM via DVE/ACT in parallel
                            if j % 2 == 0:
                                nc.vector.tensor_copy(ot, psums[j])
                            else:
                                nc.scalar.copy(ot, psums[j])
                        if final:
                            nc.scalar.dma_start(
                                out=yT[blk * 128:(blk + 1) * 128, :], in_=ot
                            )
                        elif debug:
                            nc.sync.dma_start(
                                out=dbg_d[layer][blk * 128:(blk + 1) * 128, :].bitcast(f32r),
                                in_=ot,
                            )
                        h_out.append(ot)
                h_in = h_out

    _split_excess_waits(nc)
    return nc


def prep_inputs(x, W0, W1, W2, W3, a0, a1, a2):
    """Host-side preprocessing: fold alphas+log-factor into weights, pre-tile
    weights into DMA consumption order, transpose/shard x."""
    f32 = np.float32
    x = np.asarray(x, f32)
    W = [np.array(w, f32, copy=True) for w in (W0, W1, W2, W3)]
    alphas = [np.asarray(a, f32) for a in (a0, a1, a2)]

    # fold the alpha scaling into the rows of the *next* layer's weight matrix
    for i, a in enumerate(alphas):
        W[i + 1] = a[:, None] * W[i + 1]
    # fold 1/(2pi) into the sin-segment columns of the hidden-layer weights so
    # the matmul directly produces u = z/(2pi) for the range-reduced sin path
    for i in range(3):
        n = W[i].shape[1]
        W[i][:, : (n * 2048) // 4096] *= np.float32(1.0 / (2 * np.pi))

    def retile(w):
        K, N = w.shape
        kt, ngr = K // 128, N // GW
        # blocks in consumption order: g-major, then k
        return np.ascontiguousarray(
            w.reshape(kt, 128, ngr, GW).transpose(2, 0, 1, 3).reshape(ngr * kt, 128, GW)
        )

    wt = [retile(w) for w in W]
    xT = np.ascontiguousarray(x.T)  # [d_in, B]
    in_maps = []
    for c in range(NCORES):
        shard = np.ascontiguousarray(xT[:, c * BS:(c + 1) * BS])
        in_maps.append(
            {
                "xT": shard,
                "w0": wt[0],
                "w1": wt[1],
                "w2": wt[2],
                "w3": wt[3],
            }
        )
    return in_maps


_CACHED_NC = None


def run(in_maps, trace=False, **kwargs):
    global _CACHED_NC
    from concourse import bass_utils

    bass_utils.upload_artifacts = lambda tmpdir: str(tmpdir)  # no network
    if _CACHED_NC is None:
        _CACHED_NC = build_bass()
    return bass_utils.run_bass_kernel_spmd(
        _CACHED_NC, in_maps, core_ids=list(range(NCORES)), trace=trace, **kwargs
    )


def kernel(**inputs):
    in_maps = prep_inputs(**inputs)
    res = run(in_maps, trace=False)
    y = np.concatenate(
        [np.ascontiguousarray(res.results[c]["yT"].T) for c in range(NCORES)], axis=0
    )
    return y


# revision 33
# speedup vs baseline: 1.1889x; 1.1889x over previous
"""Self-contained Trainium2 Bass kernel for nn_MixedNet_61753039781957.

MixedNet: 4-layer MLP, B=4096, D_in=1024, H=4096, D_out=1024.
  h = x
  for (W, a) in ((W0,a0),(W1,a1),(W2,a2)):
      z = h @ W
      h = a * concat([sin(z[:, :2048]), tanh(z[:, 2048:3072]), log(z[:, 3072:]**2)])
  y = h @ W3

Strategy (data-parallel, no collectives):
  - Shard batch across 8 NeuronCores (512 rows each), replicate weights.
  - Keep activations TRANSPOSED on-chip: hT[hidden, batch] so each matmul is
    psum[128(nblk), 512(batch)] += Wblk[128k, 128n].T @ hT[128k, 512] with the
    weight block as the stationary operand (no on-chip transposes anywhere).
  - Matmuls run in float32r (fp32 storage, FP22 multiply) -> full PE rate for
    moving dim >= 256.
  - alpha scaling is folded into the following layer's weight rows on the
    host; 1/(2pi) is folded into sin-segment weight columns so the on-chip sin
    path is: k = round(u) (DVE magic-number trick), f = u - k, ACT
    Sin(scale=2pi) -- the raw ACT Sin LUT is only accurate for |x| < ~3.9.
    tanh: single ACT op. log(z^2): ACT Square then ACT Ln (Square is in every
    ACT table set, so PSUM banks release before the Ln table switch).
  - Weights are pre-tiled on the host into exact DMA consumption order
    (contiguous [128, 512] blocks = 4 PSUM banks per n-group, so two groups
    double-buffer across the 8 banks); all weight DMAs issue from the SP HWDGE
    queue (the ACT queue stalls behind activation bursts), x/y use the ACT
    queue. ~10 dummy matmuls at the start keep the PE HAM clock gate warm
    through the initial DMA ramp.
    Measured: ~604us HW exec per core (2560 f32r matmuls at ~227ns issue
    spacing, ~91% of the 78.6 TF/s PE peak), rel l2 error ~2.5e-3 vs the f32
    reference (device-vs-cpu reference noise floor is ~1.9e-4).
"""

import sys
import types

sys.path.insert(0, "/opt/trn_rl_repo")

import numpy as np

NCORES = 8
B, D_IN, H, D_OUT = 4096, 1024, 4096, 1024
BS = B // NCORES  # batch shard per core
GW = 512          # n-group width (4 blocks of 128 hidden units -> 4 PSUM banks,
                  # so two groups double-buffer across the 8 PSUM banks)
SEG_SIN_END = 2048 // 128   # block idx < 16 -> sin
SEG_TANH_END = 3072 // 128  # block idx < 24 -> tanh, else ln


def _install_axon_hooks():
    """Provide antenv.axon_hooks (missing in this image) so that
    run_bass_kernel_spmd(trace=True) can capture NTFF profiles."""
    try:
        import antenv
    except ImportError:
        return
    if "antenv.axon_hooks" in sys.modules:
        return
    mod = types.ModuleType("antenv.axon_hooks")
    hook = [None]
    mod.set_axon_ntff_profile_hook = lambda h: hook.__setitem__(0, h)
    mod.get_axon_ntff_profile_hook = lambda: hook[0]
    sys.modules["antenv.axon_hooks"] = mod
    antenv.axon_hooks = mod
    try:
        from trn_agent_boot.trn_boot import _ntff_profile_via_ctypes

        h = _ntff_profile_via_ctypes("/opt/axon/libaxon_pjrt.so")
        if h is not None:
            mod.set_axon_ntff_profile_hook(h)
    except Exception:
        pass


def _patch_tile_drain():
    """walrus CoreV3 codegen rejects instructions with >4 semaphore waits; the
    TileContext tail drain collects one wait per live semaphore. Spread the
    waits over several consecutive drain instructions."""
    import concourse.tile as tile_mod
    from concourse import mybir
    from concourse.vector_clock import ScopedClock

    if getattr(tile_mod.TileContext, "_ant_drain_split", False):
        return

    MAXW = 4

    def _drain_and_barrier(self, tick_clock, wait_clock):
        nc = self.nc
        drain_inst = nc.sync.drain()
        wait_clock.add_sem_waits(
            drain_inst.ins, ScopedClock({None: tick_clock.global_clock})
        )
        si = drain_inst.ins.sync_info
        if si is not None and si.on_wait and len(si.on_wait) > MAXW:
            waits = list(si.on_wait)
            updates = list(si.on_update or [])
            drain_inst.ins.sync_info = mybir.SyncInfo(
                on_wait=waits[:MAXW], on_update=[]
            )
            rest = waits[MAXW:]
            while rest:
                chunk, rest = rest[:MAXW], rest[MAXW:]
                d = mybir.InstDrain(
                    name=nc.get_next_instruction_name(),
                    ins=[],
                    outs=[],
                    bass_is_fusable=False,
                )
                d.engine = nc.sync.engine
                d.sync_info = mybir.SyncInfo(
                    on_wait=chunk, on_update=updates if not rest else []
                )
                nc.sync.add_instruction(d)
        nc.all_engine_barrier()
        assert self.sems is not None
        popped = nc._tile_sem_poison_stack.pop()
        assert popped is self._sem_poison
        nc.clear_and_free_semaphores(list(self.sems.allocated().values()))
        nc.all_engine_barrier()

    tile_mod.TileContext._drain_and_barrier = _drain_and_barrier
    tile_mod.TileContext._ant_drain_split = True


def _split_excess_waits(nc, maxw=1, maxw_mm=1):
    """walrus CoreV3 setupSyncWait rejects instructions with too many sem
    waits (4 generally; fewer for self-loading-weights Matmult). Spill excess
    waits onto NoOps inserted just before the instruction on the same engine
    (same semantics: the engine stream is serial)."""
    from concourse import mybir

    def limit_of(inst):
        return maxw_mm if isinstance(inst, mybir.InstMatmult) else maxw

    for fn in nc.m.functions:
        for bb in fn.blocks:
            need = any(
                getattr(i, "sync_info", None)
                and i.sync_info.on_wait
                and len(i.sync_info.on_wait) > limit_of(i)
                for i in bb.instructions
            )
            if not need:
                continue
            new = []
            for inst in bb.instructions:
                lim = limit_of(inst)
                si = getattr(inst, "sync_info", None)
                if si is not None and si.on_wait and len(si.on_wait) > lim:
                    waits = list(si.on_wait)
                    head, tail = waits[:-lim] if lim else waits, waits[-lim:] if lim else []
                    while head:
                        chunk, head = head[:maxw], head[maxw:]
                        nop = mybir.InstNoOp(
                            name=nc.get_next_instruction_name(),
                            ins=[],
                            outs=[],
                            sync_info=mybir.SyncInfo(on_wait=chunk, on_update=[]),
                        )
                        nop.engine = inst.engine
                        new.append(nop)
                    inst.sync_info = mybir.SyncInfo(
                        on_wait=tail, on_update=si.on_update
                    )
                new.append(inst)
            bb.instructions = new


def build_bass(bs=BS, d_in=D_IN, h=H, d_out=D_OUT, w_bufs=16, debug=False):
    """Build the per-core Bass program (same NEFF on all cores, SPMD)."""
    _install_axon_hooks()
    _patch_tile_drain()

    import concourse.bass as bass
    import concourse.tile as tile
    from concourse import mybir

    f32 = mybir.dt.float32
    f32r = mybir.dt.float32r
    AF = mybir.ActivationFunctionType

    kt = [d_in // 128, h // 128, h // 128, h // 128]      # k tiles per layer
    ng = [h // GW, h // GW, h // GW, d_out // GW]          # n groups per layer

    nc = bass.Bass()
    xT = nc.declare_dram_parameter("xT", [d_in, bs], f32, isOutput=False)
    w_d = [
        nc.declare_dram_parameter(
            f"w{i}", [ng[i] * kt[i], 128, GW], f32, isOutput=False
        )
        for i in range(4)
    ]
    yT = nc.declare_dram_parameter("yT", [d_out, bs], f32, isOutput=True)
    dbg_d = None
    if debug:
        dbg_d = [
            nc.declare_dram_parameter(f"h{i}T", [h, bs], f32, isOutput=True)
            for i in (1, 2, 3)
        ]

    with tile.TileContext(nc) as tc:
        with (
            tc.tile_pool(name="xp", bufs=d_in // 128) as xp,
            tc.tile_pool(name="ha", bufs=h // 128) as ha,
            tc.tile_pool(name="hb", bufs=h // 128) as hb,
            tc.tile_pool(name="wp", bufs=w_bufs) as wp,
            tc.tile_pool(name="tp", bufs=4) as tp,
            tc.tile_pool(name="yp", bufs=4) as yp,
            tc.tile_pool(name="ps", bufs=8, space="PSUM") as ps,
        ):
            # Warm the PE HAM clock gate during the initial DMA ramp: the gate
            # only opens (1.2 -> 2.4 GHz) after ~3.4us of sustained PE
            # activity, so burn that time on dummy matmuls with no DMA deps.
            warm = wp.tile([128, bs], f32r, tag="warm", bufs=1)
            nc.vector.memset(warm.bitcast(f32), 1.0)
            wps = ps.tile([128, bs], f32, tag="ps")
            for i in range(10):
                nc.tensor.matmul(
                    wps, lhsT=warm[:, :128], rhs=warm,
                    start=(i == 0), stop=(i == 9),
                )

            # load x shard (transposed) into SBUF, alternating the two HWDGE
            # queues (SP + ACT) so x and the weight stream run in parallel
            h_in = []
            for k in range(d_in // 128):
                xt = xp.tile([128, bs], f32r, tag="x")
                nc.scalar.dma_start(out=xt, in_=xT[k * 128:(k + 1) * 128, :].bitcast(f32r))
                h_in.append(xt)

            def act_fun_for_block(blk, nblocks, final):
                if final:
                    return "copy"
                # segment layout scales with h for small test configs
                sin_end = (nblocks * 2048) // 4096
                tanh_end = (nblocks * 3072) // 4096
                if blk < sin_end:
                    return "sin"
                if blk < tanh_end:
                    return "tanh"
                return "ln"

            for layer in range(4):
                final = layer == 3
                out_pool = yp if final else (ha, hb, ha)[layer]
                out_tag = "y" if final else f"h{(ha, hb, ha)[layer].name}"
                h_out = []
                jn = GW // 128
                for g in range(ng[layer]):
                    psums = []
                    for j in range(jn):
                        pt = ps.tile([128, bs], f32, tag="ps", name=f"ps_l{layer}_g{g}_{j}")
                        psums.append(pt)
                    for k in range(kt[layer]):
                        wt = wp.tile([128, GW], f32r, tag="w", name=f"w_l{layer}_g{g}_k{k}")
                        # weights always via SP: the ACT engine's instruction
                        # stream stalls on activation bursts + table loads,
                        # which would delay DMA issue and starve the PE
                        nc.sync.dma_start(out=wt, in_=w_d[layer][g * kt[layer] + k, :, :].bitcast(f32r))
                        for j in range(jn):
                            nc.tensor.matmul(
                                psums[j],
                                lhsT=wt[:, j * 128:(j + 1) * 128],
                                rhs=h_in[k],
                                start=(k == 0),
                                stop=(k == kt[layer] - 1),
                            )
                    # pass 1: drain each PSUM bank ASAP with an op that is
                    # valid in ANY act table set (Square) or on DVE, so the
                    # next group's matmuls are never gated on the Ln
                    # table-load; pass 2 runs the table-set-sensitive ops.
                    pre = {}
                    for j in range(jn):
                        blk = g * jn + j
                        fun = act_fun_for_block(blk, ng[layer] * jn, final)
                        if fun == "sin":
                            MAGIC = float(np.float32(1.5 * 2 ** 23))
                            ktile = tp.tile([128, bs], f32, tag="t", name=f"k_l{layer}_b{blk}")
                            nc.vector.tensor_scalar(
                                out=ktile, in0=psums[j],
                                scalar1=MAGIC, scalar2=MAGIC,
                                op0=mybir.AluOpType.add,
                                op1=mybir.AluOpType.subtract,
                            )
                            ftile = tp.tile([128, bs], f32, tag="t2", name=f"f_l{layer}_b{blk}")
                            nc.vector.tensor_tensor(
                                out=ftile, in0=psums[j], in1=ktile,
                                op=mybir.AluOpType.subtract,
                            )
                            pre[j] = ftile
                        elif fun == "ln":
                            tt = tp.tile([128, bs], f32, tag="t", name=f"t_l{layer}_b{blk}")
                            nc.scalar.activation(tt, psums[j], AF.Square)
                            pre[j] = tt
                    for j in range(jn):
                        blk = g * jn + j
                        fun = act_fun_for_block(blk, ng[layer] * jn, final)
                        ot = out_pool.tile(
                            [128, bs], f32 if final else f32r, tag=out_tag,
                            name=f"o_l{layer}_b{blk}"
                        )
                        if fun == "sin":
                            # psum held u = z/(2pi) (folded into the weight
                            # columns on the host); pre[j] = u - round(u),
                            # so sin(2pi*pre[j]) = sin(z).
                            nc.scalar.activation(
                                ot, pre[j], AF.Sin, scale=float(2 * np.pi)
                            )
                        elif fun == "tanh":
                            nc.scalar.activation(ot, psums[j], AF.Tanh)
                        elif fun == "ln":
                            nc.scalar.activation(ot, pre[j], AF.Ln)
                        else:  # final layer: drain PSUM via DVE/ACT in parallel
                            if j % 2 == 0:
                                nc.vector.tensor_copy(ot, psums[j])
                            else:
                                nc.scalar.copy(ot, psums[j])
                        if final:
                            nc.scalar.dma_start(
                                out=yT[blk * 128:(blk + 1) * 128, :], in_=ot
                            )
                        elif debug:
                            nc.sync.dma_start(
                                out=dbg_d[layer][blk * 128:(blk + 1) * 128, :].bitcast(f32r),
                                in_=ot,
                            )
                        h_out.append(ot)
                h_in = h_out

    _split_excess_waits(nc)
    return nc


def prep_inputs(x, W0, W1, W2, W3, a0, a1, a2):
    """Host-side preprocessing: fold alphas+log-factor into weights, pre-tile
    weights into DMA consumption order, transpose/shard x."""
    f32 = np.float32
    x = np.asarray(x, f32)
    W = [np.array(w, f32, copy=True) for w in (W0, W1, W2, W3)]
    alphas = [np.asarray(a, f32) for a in (a0, a1, a2)]

    # fold the alpha scaling into the rows of the *next* layer's weight matrix
    for i, a in enumerate(alphas):
        W[i + 1] = a[:, None] * W[i + 1]
    # fold 1/(2pi) into the sin-segment columns of the hidden-layer weights so
    # the matmul directly produces u = z/(2pi) for the range-reduced sin path
    for i in range(3):
        n = W[i].shape[1]
        W[i][:, : (n * 2048) // 4096] *= np.float32(1.0 / (2 * np.pi))

    def retile(w):
        K, N = w.shape
        kt, ngr = K // 128, N // GW
        # blocks in consumption order: g-major, then k
        return np.ascontiguousarray(
            w.reshape(kt, 128, ngr, GW).transpose(2, 0, 1, 3).reshape(ngr * kt, 128, GW)
        )

    wt = [retile(w) for w in W]
    xT = np.ascontiguousarray(x.T)  # [d_in, B]
    in_maps = []
    for c in range(NCORES):
        shard = np.ascontiguousarray(xT[:, c * BS:(c + 1) * BS])
        in_maps.append(
            {
                "xT": shard,
                "w0": wt[0],
                "w1": wt[1],
                "w2": wt[2],
                "w3": wt[3],
            }
        )
    return in_maps


_CACHED_NC = None


def run(in_maps, trace=False, **kwargs):
    global _CACHED_NC
    from concourse import bass_utils

    bass_utils.upload_artifacts = lambda tmpdir: str(tmpdir)  # no network
    if _CACHED_NC is None:
        _CACHED_NC = build_bass()
    return bass_utils.run_bass_kernel_spmd(
        _CACHED_NC, in_maps, core_ids=list(range(NCORES)), trace=trace, **kwargs
    )


def kernel(**inputs):
    in_maps = prep_inputs(**inputs)
    res = run(in_maps, trace=False)
    y = np.concatenate(
        [np.ascontiguousarray(res.results[c]["yT"].T) for c in range(NCORES)], axis=0
    )
    return y
